# revision 35
# baseline (speedup 1.0000x reference)
"""Trainium2 Bass kernel for a 3-layer GIN-style GNN (nn_BaseGNN).

Sharding: data-parallel over nodes/edges by dst-owner across 8 NeuronCores.
Aggregation = one-hot matmuls over 128-edge chunks (PSUM accumulate), edge
source rows fetched from a replicated bf16 node-major feature table in DRAM
via gpsimd dma_gather.  The table is a single merged [N, 256] tensor (row ==
global node id); gathers run in two passes split by node-id PARITY through
even/odd strided views (elem_step=512) so int16 gather indices (= node//2)
cover all 50k rows.  Within each (pass, window) segment edge slots are sorted
by table row for DRAM locality; the slot->dst map is data (one-hot columns),
not program structure.  MLP/BatchNorm/pool run feature-major.  Table
re-replication is ONE ncfw AllGather per layer transition (3.2 MB/core in,
25.6 MB out); BN stats via small AllReduce; graph mean-pool partials placed
(scaled by 1/cnt) into a global buffer, AllReduced once; MLP head replicated.

All instruction streams are identical across cores (SPMD); per-core
variation lives exclusively in input data (index streams, one-hot operand
streams, pooling placement).
"""

import numpy as np
import ml_dtypes

import concourse.bass as bass
import concourse.bacc as bacc
import concourse.mybir as mybir
import concourse.tile as tile
from concourse.masks import make_identity

BF16 = mybir.dt.bfloat16
FP32 = mybir.dt.float32
I16 = mybir.dt.int16
AF = mybir.ActivationFunctionType
ALU = mybir.AluOpType

N_CORES = 8
GI_CHUNKS = 64          # 128-edge chunks per dma_gather instruction
CB = 16                 # one-hot chunks per batched is_equal
N_SWDGE_QUEUES = 2
DD_SCRATCH = 16384      # SWDGE descriptor ring bytes/partition
BN_EPS = 1e-5


def _cdiv(a, b):
    return (a + b - 1) // b


class Plan:
    pass


# ==================================================================== host
def preprocess(x, edge_index, batch, num_graphs, W_enc, b_enc, W1, b1, W2, b2,
               gamma, beta, W_fc1, b_fc1, W_fc2, b_fc2):
    p = Plan()
    N, F_IN = x.shape
    D = W_enc.shape[1]
    L = W1.shape[0]
    G = int(num_graphs)
    E = edge_index.shape[1]
    C = N_CORES
    assert N % C == 0
    NC = N // C
    assert NC % 2 == 0
    HALF = NC // 2
    W = _cdiv(NC, 128)
    p.N, p.F_IN, p.D, p.L, p.G, p.E = N, F_IN, D, L, G, E
    p.NC, p.HALF, p.W = NC, HALF, W
    assert D == 256 and F_IN == 128, "layout hardcodes D=256, F_IN=128"
    assert N // 2 < 32768, "int16 gather index range (parity-split)"

    src = np.asarray(edge_index[0], np.int64)
    dst = np.asarray(edge_index[1], np.int64)
    batch = np.asarray(batch, np.int64)

    owner = dst // NC
    # merged node-major table: row == global node id; gather passes split by
    # parity (int16 idx = src // 2 into an even/odd strided table view)
    src_half = (src % 2).astype(np.int64)
    table_row = src // 2
    dst_local = dst - owner * NC
    win = dst_local // 128

    counts = np.zeros((C, 2, W), np.int64)
    np.add.at(counts, (owner, src_half, win), 1)
    k_fix = _cdiv(counts, 128).max(axis=0)          # [2, W]
    p.k_fix = k_fix
    K_pass = k_fix.sum(axis=1).astype(np.int64)
    p.K_pass = K_pass
    p.Ktot = int(K_pass.sum())

    # within each (owner, half, window) segment, order edges by table row so
    # gather descriptors hit ascending DRAM rows (HBM locality); the slot->dst
    # map is data (dstloc one-hots), so slot order is free.
    order = np.lexsort((table_row, win, src_half, owner))
    so_owner = owner[order]
    so_half = src_half[order]
    so_win = win[order]
    so_row = table_row[order]
    so_dstloc = (dst_local - win * 128)[order]

    # chunk slot base per (p, w) in each pass stream
    slot_base = np.zeros((2, W), np.int64)
    for ph in range(2):
        b = 0
        for w in range(W):
            slot_base[ph, w] = b
            b += int(k_fix[ph, w]) * 128

    # per-(c,p,w) edge segment boundaries in the sorted arrays
    seg = np.zeros((C, 2, W, 2), np.int64)
    keys = ((so_owner * 2 + so_half) * W + so_win)
    bounds = np.searchsorted(keys, np.arange(C * 2 * W + 1))
    for c in range(C):
        for ph in range(2):
            for w in range(W):
                kk = (c * 2 + ph) * W + w
                seg[c, ph, w] = bounds[kk], bounds[kk + 1]

    def wrap16(lin):
        S = lin.shape[0] // 16
        t = lin.reshape(S, 16).T
        return np.ascontiguousarray(np.tile(t, (8, 1)).astype(np.int16))

    cnt = np.bincount(batch, minlength=G).astype(np.float64)
    cnt_inv = (1.0 / np.maximum(cnt, 1.0)).astype(np.float32)

    xf = np.asarray(x, np.float32)
    p.per_core = []
    for c in range(C):
        d = {}
        for ph in range(2):
            K = int(K_pass[ph])
            idx = np.zeros((K * 128,), np.int16)
            dl = np.full((K * 128,), -1.0, np.float32)
            for w in range(W):
                a, b = seg[c, ph, w]
                n = b - a
                sb = int(slot_base[ph, w])
                idx[sb:sb + n] = so_row[a:b].astype(np.int16)
                dl[sb:sb + n] = so_dstloc[a:b].astype(np.float32)
            d[f"idx{ph}"] = wrap16(idx)
            if ph == 0:
                dl0 = dl
            else:
                dl = np.concatenate([dl0, dl])
        d["dstloc"] = np.ascontiguousarray(
            dl.reshape(p.Ktot, 128).T).astype(ml_dtypes.bfloat16)
        nb = batch[c * NC:(c + 1) * NC]
        g_lo = int(nb[0])
        span = int(nb[-1]) - g_lo + 1
        assert span <= 128, f"core {c} spans {span} graphs"
        bl = np.full((W * 128,), -1.0, np.float32)
        bl[:NC] = (nb - g_lo).astype(np.float32)
        d["batchloc"] = np.ascontiguousarray(
            bl.reshape(W, 128).T).astype(ml_dtypes.bfloat16)
        pl = np.zeros((128, 512), np.float32)
        hi = min(128, G - g_lo)
        pl[np.arange(hi), g_lo + np.arange(hi)] = cnt_inv[g_lo:g_lo + hi]
        d["placem"] = pl.astype(ml_dtypes.bfloat16)
        d["xTown"] = np.ascontiguousarray(
            xf[c * NC:(c + 1) * NC].T).astype(ml_dtypes.bfloat16)
        p.per_core.append(d)

    sh = {}
    sh["iota"] = np.tile(np.arange(128, dtype=np.float32),
                         (128, CB)).astype(ml_dtypes.bfloat16)
    sh["wenc"] = np.asarray(W_enc, np.float32).astype(ml_dtypes.bfloat16)
    w1c = np.zeros((128, L * 4 * 128), np.float32)
    w2c = np.zeros((128, L * 4 * 128), np.float32)
    for l in range(L):
        for k in range(2):
            for m in range(2):
                col = ((l * 2 + k) * 2 + m) * 128
                w1c[:, col:col + 128] = W1[l, 128 * k:128 * (k + 1), 128 * m:128 * (m + 1)]
                w2c[:, col:col + 128] = W2[l, 128 * k:128 * (k + 1), 128 * m:128 * (m + 1)]
    sh["w1"] = w1c.astype(ml_dtypes.bfloat16)
    sh["w2"] = w2c.astype(ml_dtypes.bfloat16)

    def col2(v):
        return np.ascontiguousarray(np.asarray(v, np.float32).reshape(2, 128).T)

    sh["bencr"] = np.tile(np.asarray(b_enc, np.float32)[None, :], (128, 1))
    sh["bencc"] = col2(b_enc)
    sh["b1"] = np.concatenate([col2(b1[l]) for l in range(L)], axis=1)
    sh["b2"] = np.concatenate([col2(b2[l]) for l in range(L)], axis=1)
    sh["gam"] = np.concatenate([col2(gamma[l]) for l in range(L)], axis=1)
    sh["bet"] = np.concatenate([col2(beta[l]) for l in range(L)], axis=1)
    DL = D * L
    sh["wfc1"] = np.ascontiguousarray(
        np.asarray(W_fc1, np.float32).reshape(DL // 128, 128, 128)
        .transpose(1, 0, 2).reshape(128, DL))
    sh["wfc2"] = np.asarray(W_fc2, np.float32).reshape(128, 1)
    sh["bfc1"] = np.asarray(b_fc1, np.float32).reshape(128, 1)
    sh["bfc2"] = np.asarray(b_fc2, np.float32).reshape(1, 1)
    p.shared = sh
    return p


# ================================================================= program
def build_program(p, reps=0, skip_coll=False, ablate=(), straight=False):
    ablate = frozenset(ablate)
    C = N_CORES
    N, D, L, W, NC, HALF = p.N, p.D, p.L, p.W, p.NC, p.HALF
    NROW = C * HALF
    K_pass, k_fix, Ktot = p.K_pass, p.k_fix, p.Ktot
    NCH = _cdiv(NC, 128)
    MJ = _cdiv(NC, 512)
    DL = D * L

    nc = bacc.Bacc("TRN2", target_bir_lowering=False, debug=False,
                   enable_asserts=False, num_devices=C,
                   num_swdge_queues=N_SWDGE_QUEUES,
                   dynamic_dma_scratch_size=DD_SCRATCH)

    ein = {}

    def EIN(name, shape, dt):
        ein[name] = nc.dram_tensor(name, list(shape), dt, kind="ExternalInput").ap()
        return ein[name]

    xTown = EIN("xTown", [128, NC], BF16)
    idx_in = [EIN("idx0", [128, int(K_pass[0]) * 8], I16),
              EIN("idx1", [128, int(K_pass[1]) * 8], I16)]
    dstloc_in = EIN("dstloc", [128, Ktot], BF16)
    batchloc_in = EIN("batchloc", [128, W], BF16)
    placem_in = EIN("placem", [128, 512], BF16)
    iota_in = EIN("iota", [128, CB * 128], BF16)
    wenc_in = EIN("wenc", [128, 256], BF16)
    w1_in = EIN("w1", [128, L * 4 * 128], BF16)
    w2_in = EIN("w2", [128, L * 4 * 128], BF16)
    bencr_in = EIN("bencr", [128, 256], FP32)
    bencc_in = EIN("bencc", [128, 2], FP32)
    b1_in = EIN("b1", [128, L * 2], FP32)
    b2_in = EIN("b2", [128, L * 2], FP32)
    gam_in = EIN("gam", [128, L * 2], FP32)
    bet_in = EIN("bet", [128, L * 2], FP32)
    wfc1_in = EIN("wfc1", [128, DL], FP32)
    wfc2_in = EIN("wfc2", [128, 1], FP32)
    bfc1_in = EIN("bfc1", [128, 1], FP32)
    bfc2_in = EIN("bfc2", [1, 1], FP32)

    out_d = nc.dram_tensor("out", [1, 512], FP32, kind="ExternalOutput").ap()

    tableM = nc.dram_tensor("tableM", [N, 256], BF16, kind="Internal",
                            addr_space="Shared").ap()
    sliceM = nc.dram_tensor("sliceM", [NC, 256], BF16, kind="Internal").ap()
    viewM = tableM.rearrange("(r two) d -> r (two d)", two=2)
    arbn_in = nc.dram_tensor("arbn_in", [128, 4], FP32, kind="Internal").ap()
    arbn_out = nc.dram_tensor("arbn_out", [128, 4], FP32, kind="Internal",
                              addr_space="Shared").ap()
    arp_in = nc.dram_tensor("arp_in", [DL, 512], FP32, kind="Internal").ap()
    arp_out = nc.dram_tensor("arp_out", [DL, 512], FP32, kind="Internal",
                             addr_space="Shared").ap()

    RG = [list(range(C))]
    half_t = [viewM[:, 0:256], viewM[:, 256:512]]

    import contextlib
    with tile.TileContext(nc) as tc, contextlib.ExitStack() as ctx:
        consts = ctx.enter_context(tc.tile_pool(name="consts", bufs=1))
        gpool_s = ctx.enter_context(tc.tile_pool(name="gP", bufs=2))
        gpool = [gpool_s, gpool_s]
        ohpool = ctx.enter_context(tc.tile_pool(name="oh", bufs=3))
        hpool = ctx.enter_context(tc.tile_pool(name="h", bufs=1))
        zpool = ctx.enter_context(tc.tile_pool(name="z", bufs=1))
        spool = ctx.enter_context(tc.tile_pool(name="s", bufs=2))
        tpool = ctx.enter_context(tc.tile_pool(name="t", bufs=2))
        ps_agg = ctx.enter_context(tc.tile_pool(name="ps_agg", bufs=4, space="PSUM"))
        ps_mlp = ctx.enter_context(tc.tile_pool(name="ps_mlp", bufs=2, space="PSUM"))
        ps_msc = ctx.enter_context(tc.tile_pool(name="ps_msc", bufs=2, space="PSUM"))

        def load_const(apin, shape, dt):
            t = consts.tile(shape, dt, name="c_" + apin.tensor.name)
            nc.sync.dma_start(t[:], apin[:])
            return t

        iota_sb = load_const(iota_in, [128, CB * 128], BF16)
        wenc_sb = load_const(wenc_in, [128, 256], BF16)
        w1_sb = load_const(w1_in, [128, L * 4 * 128], BF16)
        w2_sb = load_const(w2_in, [128, L * 4 * 128], BF16)
        bencr_sb = load_const(bencr_in, [128, 256], FP32)
        bencc_sb = load_const(bencc_in, [128, 2], FP32)
        b1_sb = load_const(b1_in, [128, L * 2], FP32)
        b2_sb = load_const(b2_in, [128, L * 2], FP32)
        gam_sb = load_const(gam_in, [128, L * 2], FP32)
        bet_sb = load_const(bet_in, [128, L * 2], FP32)
        wfc1_sb = load_const(wfc1_in, [128, DL], FP32)
        wfc2_sb = load_const(wfc2_in, [128, 1], FP32)
        bfc1_sb = load_const(bfc1_in, [128, 1], FP32)
        bfc2_sb = load_const(bfc2_in, [1, 1], FP32)
        dstloc_sb = load_const(dstloc_in, [128, Ktot], BF16)
        batchloc_sb = load_const(batchloc_in, [128, W], BF16)
        placem_sb = load_const(placem_in, [128, 512], BF16)
        idx_sb = [load_const(idx_in[0], [128, int(K_pass[0]) * 8], I16),
                  load_const(idx_in[1], [128, int(K_pass[1]) * 8], I16)]
        ident = consts.tile([128, 128], BF16)
        make_identity(nc, ident[:])

        def emit_body():
            gconst = None
            if "NOCONS" in ablate:
                gconst = hpool.tile([128, 256], BF16, tag="gconst", name="gconst")
                nc.vector.tensor_copy(gconst[:], iota_sb[:, :256])
            # ---------------- encoder (own nodes only; table via AllGather) -----
            h_own = [hpool.tile([128, NC], BF16, tag=f"hown{m}", name=f"hown{m}") for m in range(2)]
            with tc.tile_pool(name="xTp", bufs=2) as xTp:
                xTown_sb = xTp.tile([128, NC], BF16, tag="xTown", name="xTown_sb")
                nc.sync.dma_start(xTown_sb[:], xTown[:])
                # node-major own rows -> sliceM (encoder runs once per core)
                for li0 in range(0, NC, 128):
                    cw = min(128, NC - li0)
                    ps = ps_mlp.tile([128, 256], FP32, tag="mlp")
                    nc.tensor.matmul(ps[:cw, :], xTown_sb[:, li0:li0 + cw],
                                     wenc_sb[:], start=True, stop=True)
                    hsb = tpool.tile([128, 256], BF16, tag="enc_h")
                    nc.vector.tensor_tensor(hsb[:cw, :], ps[:cw, :],
                                            bencr_sb[:cw, :], op=ALU.add)
                    nc.sync.dma_start(sliceM[li0:li0 + cw, :], hsb[:cw, :])
                if not skip_coll:
                    nc.gpsimd.collective_compute(
                        "AllGather", ALU.bypass, replica_groups=RG,
                        ins=[sliceM.opt()], outs=[tableM.opt()])
                # own nodes, feature-major
                for jj in range(MJ):
                    lo = jj * 512
                    nw = min(512, NC - lo)
                    for m in range(2):
                        ps = ps_mlp.tile([128, 512], FP32, tag="mlp")
                        nc.tensor.matmul(ps[:, :nw], wenc_sb[:, 128 * m:128 * (m + 1)],
                                         xTown_sb[:, lo:lo + nw], start=True, stop=True)
                        nc.scalar.activation(h_own[m][:, lo:lo + nw], ps[:, :nw],
                                             AF.Identity, bias=bencc_sb[:, m:m + 1])

            # ---------------- per-layer ----------------------------------------
            n_ginst = [int(_cdiv(int(K_pass[ph]), GI_CHUNKS)) for ph in range(2)]

            def emit_gathers(ph, table_ap, g_alias=None):
                tiles = []
                K = int(K_pass[ph])
                ew = 128 if "G256B" in ablate else 256
                for i in range(n_ginst[ph]):
                    c0 = i * GI_CHUNKS
                    nch = min(GI_CHUNKS, K - c0)
                    g = gpool[ph].tile([128, GI_CHUNKS, ew], BF16, tag="g", name=f"g{ph}t")
                    gch = 1 if "gather" in ablate else nch
                    if "seqdma" in ablate:
                        src = table_ap[0:gch * 128, :].rearrange(
                            "(pp c) d -> pp c d", pp=128)
                        nc.sync.dma_start(g[:, :gch, :], src[:, :, :ew])
                    else:
                        nc.gpsimd.dma_gather(
                            g[:, :gch, :ew], table_ap[:, :ew], idx_sb[ph][:, c0 * 8:(c0 + gch) * 8],
                            num_idxs=gch * 128, num_idxs_reg=gch * 128, elem_size=ew,
                            elem_step=512,
                            single_packet=("SPKT" in ablate),
                            queue_num=(i % N_SWDGE_QUEUES),
                        )
                    tiles.append(g if g_alias is None else g_alias)
                return tiles

            def onehot_for(ci_list):
                """Build one-hot tiles for a list of combined-stream chunk ids.
                Returns dict ci -> (tile, col0)."""
                res = {}
                i = 0
                while i < len(ci_list):
                    grp = ci_list[i:i + CB]
                    # contiguous ids
                    c0, cn = grp[0], len(grp)
                    oh = ohpool.tile([128, CB * 128], BF16, tag="oh", name="oht")
                    ohn = 1 if "onehot" in ablate else cn
                    nc.vector.tensor_tensor(
                        oh[:, :ohn * 128], iota_sb[:, :ohn * 128],
                        dstloc_sb[:, c0:c0 + ohn].to_broadcast([128, ohn, 128]),
                        op=ALU.is_equal)
                    for k, ci in enumerate(grp):
                        res[ci] = (oh, k * 128)
                    i += cn
                return res

            pooled_list = []
            for l in range(L):
                if l > 0 and not skip_coll:
                    # re-replicate the node table (one merged AllGather)
                    nc.gpsimd.collective_compute(
                        "AllGather", ALU.bypass, replica_groups=RG,
                        ins=[sliceM.opt()], outs=[tableM.opt()])

                z_pre = [zpool.tile([128, NC], BF16, tag=f"zpre{m}", name=f"zpre{m}") for m in range(2)]

                gtiles_all = [emit_gathers(0, half_t[0]),
                              emit_gathers(1, half_t[1])]
                for ph in range(2):
                    gtiles = gtiles_all[ph]
                    ci_base = 0 if ph == 0 else int(K_pass[0])
                    # one-hots for the whole pass, batched in consumption order
                    oh_map = onehot_for(list(range(ci_base, ci_base + int(K_pass[ph]))))
                    sp = 0  # chunk cursor within pass stream
                    for w in range(W):
                        kf = int(k_fix[ph, w])
                        if kf == 0:
                            continue
                        lo = w * 128
                        cw = min(128, NC - lo)
                        aggs = [ps_agg.tile([128, 128], FP32, tag="agg", name=f"agg{m}")
                                for m in range(2)]
                        kf_eff = 1 if "aggmm" in ablate else kf
                        for j in range(kf_eff):
                            ci = sp + j
                            g = gtiles[ci // GI_CHUNKS]
                            if "NOCONS" in ablate:
                                gsl = gconst[:, :]
                            else:
                                gsl = g[:, ci % GI_CHUNKS, :]
                            oh, col0 = oh_map[ci_base + ci]
                            ohsl = oh[:, col0:col0 + 128]
                            for m in range(2):
                                mm = 0 if "G256B" in ablate else m
                                nc.tensor.matmul(
                                    aggs[m][:, :], gsl[:, 128 * mm:128 * (mm + 1)],
                                    ohsl, start=(j == 0), stop=(j == kf_eff - 1))
                        for m in range(2):
                            if ph == 0:
                                nc.vector.tensor_tensor(
                                    z_pre[m][:, lo:lo + cw], aggs[m][:, :cw],
                                    h_own[m][:, lo:lo + cw], op=ALU.add)
                            else:
                                nc.vector.tensor_tensor(
                                    z_pre[m][:, lo:lo + cw], aggs[m][:, :cw],
                                    z_pre[m][:, lo:lo + cw], op=ALU.add)
                        sp += kf
                    # windows with kf == 0 in pass A still need the h_own copy
                    if ph == 0:
                        for w in range(W):
                            if int(k_fix[0, w]) == 0:
                                lo = w * 128
                                cw = min(128, NC - lo)
                                for m in range(2):
                                    nc.vector.tensor_copy(z_pre[m][:, lo:lo + cw],
                                                          h_own[m][:, lo:lo + cw])

                # ---- MLP ----
                z2 = z_pre
                for jj in range(0 if "mlp" in ablate else MJ):
                    lo = jj * 512
                    nw = min(512, NC - lo)
                    z1t = [tpool.tile([128, 512], BF16, tag=f"z1_{m}", name=f"z1t{m}") for m in range(2)]
                    for m in range(2):
                        ps = ps_mlp.tile([128, 512], FP32, tag="mlp")
                        for k in range(2):
                            col = ((l * 2 + k) * 2 + m) * 128
                            nc.tensor.matmul(ps[:, :nw], w1_sb[:, col:col + 128],
                                             z_pre[k][:, lo:lo + nw],
                                             start=(k == 0), stop=(k == 1))
                        nc.scalar.activation(z1t[m][:, :nw], ps[:, :nw], AF.Relu,
                                             bias=b1_sb[:, 2 * l + m:2 * l + m + 1])
                    for m in range(2):
                        ps = ps_mlp.tile([128, 512], FP32, tag="mlp")
                        for k in range(2):
                            col = ((l * 2 + k) * 2 + m) * 128
                            nc.tensor.matmul(ps[:, :nw], w2_sb[:, col:col + 128],
                                             z1t[k][:, :nw],
                                             start=(k == 0), stop=(k == 1))
                        nc.scalar.activation(z2[m][:, lo:lo + nw], ps[:, :nw], AF.Identity,
                                             bias=b2_sb[:, 2 * l + m:2 * l + m + 1])

                # ---- BN stats + AllReduce ----
                nbch = _cdiv(NC, 512)
                stt = spool.tile([128, 4], FP32, tag="stt")
                bnacc = tpool.tile([128, nbch, 6], FP32, tag="bnacc")
                for m in range(2):
                    for jj in range(nbch):
                        lo = jj * 512
                        nw = min(512, NC - lo)
                        nc.vector.bn_stats(bnacc[:, jj, :], z2[m][:, lo:lo + nw])
                    ag = spool.tile([128, 2], FP32, tag="bnag")
                    nc.vector.bn_aggr(ag[:], bnacc[:])
                    # (mean, var) -> (mean, E[z^2])
                    sq = spool.tile([128, 1], FP32, tag="bnsq")
                    nc.vector.tensor_tensor(sq[:], ag[:, 0:1], ag[:, 0:1], op=ALU.mult)
                    nc.vector.tensor_copy(stt[:, 2 * m:2 * m + 1], ag[:, 0:1])
                    nc.vector.tensor_tensor(stt[:, 2 * m + 1:2 * m + 2], ag[:, 1:2],
                                            sq[:], op=ALU.add)
                nc.sync.dma_start(arbn_in[:], stt[:])
                if not skip_coll:
                    nc.gpsimd.collective_compute("AllReduce", ALU.add, replica_groups=RG,
                                                 ins=[arbn_in.opt()], outs=[arbn_out.opt()])
                stg = spool.tile([128, 4], FP32, tag="stg")
                nc.sync.dma_start(stg[:], arbn_out[:])
                # a = gamma * rsqrt(var + eps);  cb = beta - a * mean
                scl = spool.tile([128, 2], FP32, tag="scl")
                cbt = spool.tile([128, 2], FP32, tag="cbt")
                for m in range(2):
                    mean = spool.tile([128, 1], FP32, tag="bmean")
                    e2 = spool.tile([128, 1], FP32, tag="be2")
                    nc.vector.tensor_scalar(mean[:], stg[:, 2 * m:2 * m + 1], 0.125,
                                            None, op0=ALU.mult)
                    nc.vector.tensor_scalar(e2[:], stg[:, 2 * m + 1:2 * m + 2], 0.125,
                                            None, op0=ALU.mult)
                    var = spool.tile([128, 1], FP32, tag="bvar")
                    nc.vector.tensor_tensor(var[:], mean[:], mean[:], op=ALU.mult)
                    nc.vector.tensor_tensor(var[:], e2[:], var[:], op=ALU.subtract)
                    nc.vector.tensor_scalar(var[:], var[:], float(BN_EPS), None,
                                            op0=ALU.add)
                    std = spool.tile([128, 1], FP32, tag="bstd")
                    nc.scalar.activation(std[:], var[:], AF.Sqrt)
                    rstd = spool.tile([128, 1], FP32, tag="brstd")
                    nc.vector.reciprocal(rstd[:], std[:])
                    nc.vector.tensor_tensor(scl[:, m:m + 1], rstd[:],
                                            gam_sb[:, 2 * l + m:2 * l + m + 1], op=ALU.mult)
                    tmp = spool.tile([128, 1], FP32, tag="btmp")
                    nc.vector.tensor_tensor(tmp[:], scl[:, m:m + 1], mean[:], op=ALU.mult)
                    nc.vector.tensor_tensor(cbt[:, m:m + 1],
                                            bet_sb[:, 2 * l + m:2 * l + m + 1],
                                            tmp[:], op=ALU.subtract)

                # ---- normalize + relu -> h_next ----
                h_next = [hpool.tile([128, NC], BF16, tag=f"hown{m}", name=f"hnext{m}") for m in range(2)]
                for m in range(2):
                    for jj in range(nbch):
                        lo = jj * 512
                        nw = min(512, NC - lo)
                        nc.scalar.activation(h_next[m][:, lo:lo + nw],
                                             z2[m][:, lo:lo + nw], AF.Relu,
                                             bias=cbt[:, m:m + 1], scale=scl[:, m:m + 1])

                # ---- transpose to node-major; pooling; slice write ----
                pooled_ps = ps_msc.tile([128, 256], FP32, tag="msc")
                for j in range(NCH):
                    lo = j * 128
                    cw = min(128, NC - lo)
                    hnm = tpool.tile([128, 256], BF16, tag="hnm")
                    for m in range(2):
                        tp = ps_msc.tile([128, 128], BF16, tag="msc")
                        nc.tensor.transpose(tp[:cw, :], h_next[m][:, lo:lo + cw],
                                            ident[:])
                        nc.vector.tensor_copy(hnm[:cw, 128 * m:128 * (m + 1)],
                                              tp[:cw, :])
                    # pooling one-hot + matmul
                    po = ohpool.tile([128, 128], BF16, tag="poh")
                    nc.vector.tensor_tensor(
                        po[:], iota_sb[:, :128],
                        batchloc_sb[:, j:j + 1].to_broadcast([128, 128]),
                        op=ALU.is_equal)
                    nc.tensor.matmul(pooled_ps[:], po[:cw, :], hnm[:cw, :],
                                     start=(j == 0), stop=(j == NCH - 1))
                    if l < L - 1:
                        nc.sync.dma_start(sliceM[lo:lo + cw, :], hnm[:cw, :])
                pooled_sb = tpool.tile([128, 256], BF16, tag="pooled")
                nc.vector.tensor_copy(pooled_sb[:], pooled_ps[:])
                for m in range(2):
                    pl_ps = ps_msc.tile([128, 512], FP32, tag="msc")
                    nc.tensor.matmul(pl_ps[:], pooled_sb[:, 128 * m:128 * (m + 1)],
                                     placem_sb[:], start=True, stop=True)
                    gp = tpool.tile([128, 512], FP32, tag="gp")
                    nc.vector.tensor_copy(gp[:], pl_ps[:])
                    nc.sync.dma_start(arp_in[(l * 2 + m) * 128:(l * 2 + m + 1) * 128, :],
                                      gp[:])
                h_own = h_next

            # ---------------- pooled AllReduce + head ---------------------------
            if not skip_coll:
                nc.gpsimd.collective_compute("AllReduce", ALU.add, replica_groups=RG,
                                             ins=[arp_in.opt()], outs=[arp_out.opt()])
            y1ps = ps_mlp.tile([128, 512], FP32, tag="mlp")
            gtiles = []
            for k in range(DL // 128):
                gk = tpool.tile([128, 512], FP32, tag="gark")
                nc.sync.dma_start(gk[:], arp_out[128 * k:128 * (k + 1), :])
                gtiles.append(gk)
            for k in range(DL // 128):
                nc.tensor.matmul(y1ps[:], wfc1_sb[:, 128 * k:128 * (k + 1)],
                                 gtiles[k][:], start=(k == 0), stop=(k == DL // 128 - 1))
            y1 = tpool.tile([128, 512], FP32, tag="y1")
            nc.scalar.activation(y1[:], y1ps[:], AF.Relu, bias=bfc1_sb[:])
            y2ps = ps_msc.tile([1, 512], FP32, tag="msc")
            nc.tensor.matmul(y2ps[:], wfc2_sb[:], y1[:], start=True, stop=True)
            osb = tpool.tile([1, 512], FP32, tag="osb")
            nc.scalar.activation(osb[:], y2ps[:], AF.Identity, bias=bfc2_sb[:])
            nc.sync.dma_start(out_d[:], osb[:])

        if reps and straight:
            for _ in range(reps):
                emit_body()
        elif reps:
            with tc.For_i(0, reps, 1):
                emit_body()
        else:
            emit_body()

    nc.compile()
    return nc


# ==================================================================== run
_CACHE = {}


def _get_runner(p):
    import jax
    from jax.sharding import Mesh, PartitionSpec
    from jax.experimental.shard_map import shard_map
    from concourse.bass2jax import _bass_exec_p, install_neuronx_cc_hook

    nc = build_program(p)
    install_neuronx_cc_hook()
    part_name = nc.partition_id_tensor.name if nc.partition_id_tensor else None
    in_names, out_names, out_avals, zero_outs = [], [], [], []
    for alloc in nc.m.functions[0].allocations:
        if not isinstance(alloc, mybir.MemoryLocationSet):
            continue
        name = alloc.memorylocations[0].name
        if alloc.kind == "ExternalInput":
            if name != part_name:
                in_names.append(name)
        elif alloc.kind == "ExternalOutput":
            out_names.append(name)
            shape = tuple(alloc.tensor_shape)
            dtype = mybir.dt.np(alloc.dtype)
            out_avals.append(jax.core.ShapedArray(shape, dtype))
            zero_outs.append(np.zeros(shape, dtype))
    n_params = len(in_names)
    all_in_names = list(in_names) + list(out_names)
    if part_name is not None:
        all_in_names.append(part_name)

    def _body(*args):
        from concourse.bass2jax import partition_id_tensor
        operands = list(args)
        if part_name is not None:
            operands.append(partition_id_tensor())
        outs = _bass_exec_p.bind(
            *operands, out_avals=tuple(out_avals), in_names=tuple(all_in_names),
            out_names=tuple(out_names), lowering_input_output_aliases=(),
            sim_require_finite=False, sim_require_nnan=False, nc=nc)
        return tuple(outs)

    devices = jax.devices()[:N_CORES]
    mesh = Mesh(np.asarray(devices), ("core",))
    specs = (PartitionSpec("core"),) * (n_params + len(out_names))
    fn = jax.jit(shard_map(_body, mesh=mesh, in_specs=specs,
                           out_specs=(PartitionSpec("core"),) * len(out_names),
                           check_rep=False), keep_unused=True)
    return nc, fn, in_names, out_names, out_avals, zero_outs, mesh


def _device_args(p):
    import jax
    from jax.sharding import NamedSharding, PartitionSpec
    nc, fn, in_names, out_names, out_avals, zero_outs, mesh = _CACHE["runner"]
    per_core_maps = []
    for c in range(N_CORES):
        m = dict(p.shared)
        m.update(p.per_core[c])
        per_core_maps.append(m)
    concat_in = [np.concatenate([np.asarray(per_core_maps[c][nm])[None]
                                 for c in range(N_CORES)], axis=0)
                 .reshape(-1, *np.asarray(per_core_maps[0][nm]).shape[1:])
                 for nm in in_names]
    concat_zero = [np.zeros((N_CORES * z.shape[0], *z.shape[1:]), z.dtype)
                   for z in zero_outs]
    sh = NamedSharding(mesh, PartitionSpec("core"))
    args = [jax.device_put(a, sh) for a in concat_in + concat_zero]
    for a in args:
        a.block_until_ready()
    return args


def run_on_device(p):
    import jax
    sig = (p.N, p.E, p.G, p.Ktot, tuple(map(int, p.K_pass)),
           tuple(map(int, p.k_fix.ravel())))
    if _CACHE.get("sig") != sig:
        _CACHE.clear()
        _CACHE["sig"] = sig
    if "runner" not in _CACHE:
        _CACHE["runner"] = _get_runner(p)
    if "args" not in _CACHE:
        _CACHE["args"] = _device_args(p)
    nc, fn, in_names, out_names, out_avals, zero_outs, mesh = _CACHE["runner"]
    outs = fn(*_CACHE["args"])
    for o in outs:
        o.block_until_ready()
    res = np.asarray(outs[out_names.index("out")])
    res = res.reshape(N_CORES, 1, 512)[0, 0]     # core 0
    return res


def kernel(**inputs):
    p = preprocess(**inputs)
    _CACHE.pop("args", None)       # force fresh input upload for new data
    out = run_on_device(p)
    return out[:p.G].astype(np.float32).reshape(p.G, 1)



# revision 36
# speedup vs baseline: 1.3540x; 1.3540x over previous
"""Trainium2 Bass kernel for a 3-layer GIN-style GNN (nn_BaseGNN).

Sharding: data-parallel over nodes/edges by dst-owner across 8 NeuronCores.
Aggregation = one-hot matmuls over 128-edge chunks (PSUM accumulate), edge
source rows fetched from a replicated bf16 node-major feature table in DRAM
via gpsimd dma_gather.  The table is a single merged [N, 256] tensor (row ==
global node id); gathers run in two passes split by node-id PARITY through
even/odd strided views (elem_step=512) so int16 gather indices (= node//2)
cover all 50k rows.  Within each (pass, window) segment edge slots are sorted
by table row for DRAM locality; the slot->dst map is data (one-hot columns),
not program structure.  MLP/BatchNorm/pool run feature-major.  Table
re-replication is ONE ncfw AllGather per layer transition (3.2 MB/core in,
25.6 MB out); BN stats via small AllReduce; graph mean-pool partials placed
(scaled by 1/cnt) into a global buffer, AllReduced once; MLP head replicated.

All instruction streams are identical across cores (SPMD); per-core
variation lives exclusively in input data (index streams, one-hot operand
streams, pooling placement).
"""

import numpy as np
import ml_dtypes

import concourse.bass as bass
import concourse.bacc as bacc
import concourse.mybir as mybir
import concourse.tile as tile
from concourse.masks import make_identity

BF16 = mybir.dt.bfloat16
FP32 = mybir.dt.float32
I16 = mybir.dt.int16
AF = mybir.ActivationFunctionType
ALU = mybir.AluOpType

N_CORES = 8
GI_CHUNKS = 32          # 128-edge chunks per dma_gather instruction
CB = 16                 # one-hot chunks per batched is_equal
N_SWDGE_QUEUES = 2
DD_SCRATCH = 16384      # SWDGE descriptor ring bytes/partition
BN_EPS = 1e-5


def _cdiv(a, b):
    return (a + b - 1) // b


class Plan:
    pass


# ==================================================================== host
def preprocess(x, edge_index, batch, num_graphs, W_enc, b_enc, W1, b1, W2, b2,
               gamma, beta, W_fc1, b_fc1, W_fc2, b_fc2):
    p = Plan()
    N, F_IN = x.shape
    D = W_enc.shape[1]
    L = W1.shape[0]
    G = int(num_graphs)
    E = edge_index.shape[1]
    C = N_CORES
    assert N % C == 0
    NC = N // C
    assert NC % 2 == 0
    HALF = NC // 2
    W = _cdiv(NC, 128)
    p.N, p.F_IN, p.D, p.L, p.G, p.E = N, F_IN, D, L, G, E
    p.NC, p.HALF, p.W = NC, HALF, W
    assert D == 256 and F_IN == 128, "layout hardcodes D=256, F_IN=128"
    assert N // 2 < 32768, "int16 gather index range (parity-split)"

    src = np.asarray(edge_index[0], np.int64)
    dst = np.asarray(edge_index[1], np.int64)
    batch = np.asarray(batch, np.int64)

    owner = dst // NC
    # merged node-major table: row == global node id; gather passes split by
    # parity (int16 idx = src // 2 into an even/odd strided table view)
    src_half = (src % 2).astype(np.int64)
    table_row = src // 2
    dst_local = dst - owner * NC
    win = dst_local // 128

    counts = np.zeros((C, 2, W), np.int64)
    np.add.at(counts, (owner, src_half, win), 1)
    k_fix = _cdiv(counts, 128).max(axis=0)          # [2, W]
    p.k_fix = k_fix
    K_pass = k_fix.sum(axis=1).astype(np.int64)
    p.K_pass = K_pass
    p.Ktot = int(K_pass.sum())

    # within each (owner, half, window) segment, order edges by table row so
    # gather descriptors hit ascending DRAM rows (HBM locality); the slot->dst
    # map is data (dstloc one-hots), so slot order is free.
    order = np.lexsort((table_row, win, src_half, owner))
    so_owner = owner[order]
    so_half = src_half[order]
    so_win = win[order]
    so_row = table_row[order]
    so_dstloc = (dst_local - win * 128)[order]

    # chunk slot base per (p, w) in each pass stream
    slot_base = np.zeros((2, W), np.int64)
    for ph in range(2):
        b = 0
        for w in range(W):
            slot_base[ph, w] = b
            b += int(k_fix[ph, w]) * 128

    # per-(c,p,w) edge segment boundaries in the sorted arrays
    seg = np.zeros((C, 2, W, 2), np.int64)
    keys = ((so_owner * 2 + so_half) * W + so_win)
    bounds = np.searchsorted(keys, np.arange(C * 2 * W + 1))
    for c in range(C):
        for ph in range(2):
            for w in range(W):
                kk = (c * 2 + ph) * W + w
                seg[c, ph, w] = bounds[kk], bounds[kk + 1]

    def wrap16(lin):
        S = lin.shape[0] // 16
        t = lin.reshape(S, 16).T
        return np.ascontiguousarray(np.tile(t, (8, 1)).astype(np.int16))

    cnt = np.bincount(batch, minlength=G).astype(np.float64)
    cnt_inv = (1.0 / np.maximum(cnt, 1.0)).astype(np.float32)

    xf = np.asarray(x, np.float32)
    p.per_core = []
    for c in range(C):
        d = {}
        for ph in range(2):
            K = int(K_pass[ph])
            idx = np.zeros((K * 128,), np.int16)
            dl = np.full((K * 128,), -1.0, np.float32)
            for w in range(W):
                a, b = seg[c, ph, w]
                n = b - a
                sb = int(slot_base[ph, w])
                idx[sb:sb + n] = so_row[a:b].astype(np.int16)
                dl[sb:sb + n] = so_dstloc[a:b].astype(np.float32)
            d[f"idx{ph}"] = wrap16(idx)
            if ph == 0:
                dl0 = dl
            else:
                dl = np.concatenate([dl0, dl])
        d["dstloc"] = np.ascontiguousarray(
            dl.reshape(p.Ktot, 128).T).astype(ml_dtypes.bfloat16)
        nb = batch[c * NC:(c + 1) * NC]
        g_lo = int(nb[0])
        span = int(nb[-1]) - g_lo + 1
        assert span <= 128, f"core {c} spans {span} graphs"
        bl = np.full((W * 128,), -1.0, np.float32)
        bl[:NC] = (nb - g_lo).astype(np.float32)
        d["batchloc"] = np.ascontiguousarray(
            bl.reshape(W, 128).T).astype(ml_dtypes.bfloat16)
        pl = np.zeros((128, 512), np.float32)
        hi = min(128, G - g_lo)
        pl[np.arange(hi), g_lo + np.arange(hi)] = cnt_inv[g_lo:g_lo + hi]
        d["placem"] = pl.astype(ml_dtypes.bfloat16)
        d["xTown"] = np.ascontiguousarray(
            xf[c * NC:(c + 1) * NC].T).astype(ml_dtypes.bfloat16)
        p.per_core.append(d)

    sh = {}
    sh["iota"] = np.tile(np.arange(128, dtype=np.float32),
                         (128, CB)).astype(ml_dtypes.bfloat16)
    sh["wenc"] = np.asarray(W_enc, np.float32).astype(ml_dtypes.bfloat16)
    w1c = np.zeros((128, L * 4 * 128), np.float32)
    w2c = np.zeros((128, L * 4 * 128), np.float32)
    for l in range(L):
        for k in range(2):
            for m in range(2):
                col = ((l * 2 + k) * 2 + m) * 128
                w1c[:, col:col + 128] = W1[l, 128 * k:128 * (k + 1), 128 * m:128 * (m + 1)]
                w2c[:, col:col + 128] = W2[l, 128 * k:128 * (k + 1), 128 * m:128 * (m + 1)]
    sh["w1"] = w1c.astype(ml_dtypes.bfloat16)
    sh["w2"] = w2c.astype(ml_dtypes.bfloat16)

    def col2(v):
        return np.ascontiguousarray(np.asarray(v, np.float32).reshape(2, 128).T)

    sh["bencr"] = np.tile(np.asarray(b_enc, np.float32)[None, :], (128, 1))
    sh["bencc"] = col2(b_enc)
    sh["b1"] = np.concatenate([col2(b1[l]) for l in range(L)], axis=1)
    sh["b2"] = np.concatenate([col2(b2[l]) for l in range(L)], axis=1)
    sh["gam"] = np.concatenate([col2(gamma[l]) for l in range(L)], axis=1)
    sh["bet"] = np.concatenate([col2(beta[l]) for l in range(L)], axis=1)
    DL = D * L
    sh["wfc1"] = np.ascontiguousarray(
        np.asarray(W_fc1, np.float32).reshape(DL // 128, 128, 128)
        .transpose(1, 0, 2).reshape(128, DL))
    sh["wfc2"] = np.asarray(W_fc2, np.float32).reshape(128, 1)
    sh["bfc1"] = np.asarray(b_fc1, np.float32).reshape(128, 1)
    sh["bfc2"] = np.asarray(b_fc2, np.float32).reshape(1, 1)
    p.shared = sh
    return p


# ================================================================= program
def build_program(p, reps=0, skip_coll=False, ablate=(), straight=False):
    ablate = frozenset(ablate)
    C = N_CORES
    N, D, L, W, NC, HALF = p.N, p.D, p.L, p.W, p.NC, p.HALF
    NROW = C * HALF
    K_pass, k_fix, Ktot = p.K_pass, p.k_fix, p.Ktot
    NCH = _cdiv(NC, 128)
    MJ = _cdiv(NC, 512)
    DL = D * L

    nc = bacc.Bacc("TRN2", target_bir_lowering=False, debug=False,
                   enable_asserts=False, num_devices=C,
                   num_swdge_queues=N_SWDGE_QUEUES,
                   dynamic_dma_scratch_size=DD_SCRATCH)

    ein = {}

    def EIN(name, shape, dt):
        ein[name] = nc.dram_tensor(name, list(shape), dt, kind="ExternalInput").ap()
        return ein[name]

    xTown = EIN("xTown", [128, NC], BF16)
    idx_in = [EIN("idx0", [128, int(K_pass[0]) * 8], I16),
              EIN("idx1", [128, int(K_pass[1]) * 8], I16)]
    dstloc_in = EIN("dstloc", [128, Ktot], BF16)
    batchloc_in = EIN("batchloc", [128, W], BF16)
    placem_in = EIN("placem", [128, 512], BF16)
    iota_in = EIN("iota", [128, CB * 128], BF16)
    wenc_in = EIN("wenc", [128, 256], BF16)
    w1_in = EIN("w1", [128, L * 4 * 128], BF16)
    w2_in = EIN("w2", [128, L * 4 * 128], BF16)
    bencr_in = EIN("bencr", [128, 256], FP32)
    bencc_in = EIN("bencc", [128, 2], FP32)
    b1_in = EIN("b1", [128, L * 2], FP32)
    b2_in = EIN("b2", [128, L * 2], FP32)
    gam_in = EIN("gam", [128, L * 2], FP32)
    bet_in = EIN("bet", [128, L * 2], FP32)
    wfc1_in = EIN("wfc1", [128, DL], FP32)
    wfc2_in = EIN("wfc2", [128, 1], FP32)
    bfc1_in = EIN("bfc1", [128, 1], FP32)
    bfc2_in = EIN("bfc2", [1, 1], FP32)

    out_d = nc.dram_tensor("out", [1, 512], FP32, kind="ExternalOutput").ap()

    tableM = nc.dram_tensor("tableM", [N, 256], BF16, kind="Internal",
                            addr_space="Shared").ap()
    sliceM = nc.dram_tensor("sliceM", [NC, 256], BF16, kind="Internal").ap()
    viewM = tableM.rearrange("(r two) d -> r (two d)", two=2)
    arbn_in = nc.dram_tensor("arbn_in", [128, 4], FP32, kind="Internal").ap()
    arbn_out = nc.dram_tensor("arbn_out", [128, 4], FP32, kind="Internal",
                              addr_space="Shared").ap()
    arp_in = nc.dram_tensor("arp_in", [DL, 512], FP32, kind="Internal").ap()
    arp_out = nc.dram_tensor("arp_out", [DL, 512], FP32, kind="Internal",
                             addr_space="Shared").ap()

    RG = [list(range(C))]
    half_t = [viewM[:, 0:256], viewM[:, 256:512]]

    import contextlib
    with tile.TileContext(nc) as tc, contextlib.ExitStack() as ctx:
        consts = ctx.enter_context(tc.tile_pool(name="consts", bufs=1))
        gpool_s = ctx.enter_context(tc.tile_pool(name="gP", bufs=3))
        gpool = [gpool_s, gpool_s]
        ohpool = ctx.enter_context(tc.tile_pool(name="oh", bufs=3))
        hpool = ctx.enter_context(tc.tile_pool(name="h", bufs=1))
        zpool = ctx.enter_context(tc.tile_pool(name="z", bufs=1))
        spool = ctx.enter_context(tc.tile_pool(name="s", bufs=2))
        tpool = ctx.enter_context(tc.tile_pool(name="t", bufs=2))
        ps_agg = ctx.enter_context(tc.tile_pool(name="ps_agg", bufs=4, space="PSUM"))
        ps_mlp = ctx.enter_context(tc.tile_pool(name="ps_mlp", bufs=2, space="PSUM"))
        ps_msc = ctx.enter_context(tc.tile_pool(name="ps_msc", bufs=2, space="PSUM"))

        def load_const(apin, shape, dt):
            t = consts.tile(shape, dt, name="c_" + apin.tensor.name)
            nc.sync.dma_start(t[:], apin[:])
            return t

        iota_sb = load_const(iota_in, [128, CB * 128], BF16)
        wenc_sb = load_const(wenc_in, [128, 256], BF16)
        w1_sb = load_const(w1_in, [128, L * 4 * 128], BF16)
        w2_sb = load_const(w2_in, [128, L * 4 * 128], BF16)
        bencr_sb = load_const(bencr_in, [128, 256], FP32)
        bencc_sb = load_const(bencc_in, [128, 2], FP32)
        b1_sb = load_const(b1_in, [128, L * 2], FP32)
        b2_sb = load_const(b2_in, [128, L * 2], FP32)
        gam_sb = load_const(gam_in, [128, L * 2], FP32)
        bet_sb = load_const(bet_in, [128, L * 2], FP32)
        wfc1_sb = load_const(wfc1_in, [128, DL], FP32)
        wfc2_sb = load_const(wfc2_in, [128, 1], FP32)
        bfc1_sb = load_const(bfc1_in, [128, 1], FP32)
        bfc2_sb = load_const(bfc2_in, [1, 1], FP32)
        dstloc_sb = load_const(dstloc_in, [128, Ktot], BF16)
        batchloc_sb = load_const(batchloc_in, [128, W], BF16)
        placem_sb = load_const(placem_in, [128, 512], BF16)
        idx_sb = [load_const(idx_in[0], [128, int(K_pass[0]) * 8], I16),
                  load_const(idx_in[1], [128, int(K_pass[1]) * 8], I16)]
        ident = consts.tile([128, 128], BF16)
        make_identity(nc, ident[:])

        def emit_body():
            gconst = None
            if "NOCONS" in ablate:
                gconst = hpool.tile([128, 256], BF16, tag="gconst", name="gconst")
                nc.vector.tensor_copy(gconst[:], iota_sb[:, :256])
            # ---------------- encoder (own nodes only; table via AllGather) -----
            h_own = [hpool.tile([128, NC], BF16, tag=f"hown{m}", name=f"hown{m}") for m in range(2)]
            with tc.tile_pool(name="xTp", bufs=2) as xTp:
                xTown_sb = xTp.tile([128, NC], BF16, tag="xTown", name="xTown_sb")
                nc.sync.dma_start(xTown_sb[:], xTown[:])
                # node-major own rows -> sliceM (encoder runs once per core)
                for li0 in range(0, NC, 128):
                    cw = min(128, NC - li0)
                    ps = ps_mlp.tile([128, 256], FP32, tag="mlp")
                    nc.tensor.matmul(ps[:cw, :], xTown_sb[:, li0:li0 + cw],
                                     wenc_sb[:], start=True, stop=True)
                    hsb = tpool.tile([128, 256], BF16, tag="enc_h")
                    nc.vector.tensor_tensor(hsb[:cw, :], ps[:cw, :],
                                            bencr_sb[:cw, :], op=ALU.add)
                    nc.sync.dma_start(sliceM[li0:li0 + cw, :], hsb[:cw, :])
                if not skip_coll:
                    nc.gpsimd.collective_compute(
                        "AllGather", ALU.bypass, replica_groups=RG,
                        ins=[sliceM.opt()], outs=[tableM.opt()])
                # own nodes, feature-major
                for jj in range(MJ):
                    lo = jj * 512
                    nw = min(512, NC - lo)
                    for m in range(2):
                        ps = ps_mlp.tile([128, 512], FP32, tag="mlp")
                        nc.tensor.matmul(ps[:, :nw], wenc_sb[:, 128 * m:128 * (m + 1)],
                                         xTown_sb[:, lo:lo + nw], start=True, stop=True)
                        nc.scalar.activation(h_own[m][:, lo:lo + nw], ps[:, :nw],
                                             AF.Identity, bias=bencc_sb[:, m:m + 1])

            # ---------------- per-layer ----------------------------------------
            n_ginst = [int(_cdiv(int(K_pass[ph]), GI_CHUNKS)) for ph in range(2)]

            def emit_gathers(ph, table_ap, g_alias=None):
                tiles = []
                K = int(K_pass[ph])
                ew = 128 if "G256B" in ablate else 256
                for i in range(n_ginst[ph]):
                    c0 = i * GI_CHUNKS
                    nch = min(GI_CHUNKS, K - c0)
                    g = gpool[ph].tile([128, GI_CHUNKS, ew], BF16, tag="g", name=f"g{ph}t")
                    gch = 1 if "gather" in ablate else nch
                    if "seqdma" in ablate:
                        src = table_ap[0:gch * 128, :].rearrange(
                            "(pp c) d -> pp c d", pp=128)
                        nc.sync.dma_start(g[:, :gch, :], src[:, :, :ew])
                    else:
                        nc.gpsimd.dma_gather(
                            g[:, :gch, :ew], table_ap[:, :ew], idx_sb[ph][:, c0 * 8:(c0 + gch) * 8],
                            num_idxs=gch * 128, num_idxs_reg=gch * 128, elem_size=ew,
                            elem_step=512,
                            single_packet=("SPKT" in ablate),
                            queue_num=(i % N_SWDGE_QUEUES),
                        )
                    tiles.append(g if g_alias is None else g_alias)
                return tiles

            def onehot_for(ci_list):
                """Build one-hot tiles for a list of combined-stream chunk ids.
                Returns dict ci -> (tile, col0)."""
                res = {}
                i = 0
                while i < len(ci_list):
                    grp = ci_list[i:i + CB]
                    # contiguous ids
                    c0, cn = grp[0], len(grp)
                    oh = ohpool.tile([128, CB * 128], BF16, tag="oh", name="oht")
                    ohn = 1 if "onehot" in ablate else cn
                    nc.vector.tensor_tensor(
                        oh[:, :ohn * 128], iota_sb[:, :ohn * 128],
                        dstloc_sb[:, c0:c0 + ohn].to_broadcast([128, ohn, 128]),
                        op=ALU.is_equal)
                    for k, ci in enumerate(grp):
                        res[ci] = (oh, k * 128)
                    i += cn
                return res

            pooled_list = []
            for l in range(L):
                if l > 0 and not skip_coll:
                    # re-replicate the node table (one merged AllGather)
                    nc.gpsimd.collective_compute(
                        "AllGather", ALU.bypass, replica_groups=RG,
                        ins=[sliceM.opt()], outs=[tableM.opt()])

                z_pre = [zpool.tile([128, NC], BF16, tag=f"zpre{m}", name=f"zpre{m}") for m in range(2)]

                gtiles_all = [emit_gathers(0, half_t[0]),
                              emit_gathers(1, half_t[1])]
                for ph in range(2):
                    gtiles = gtiles_all[ph]
                    ci_base = 0 if ph == 0 else int(K_pass[0])
                    # one-hots for the whole pass, batched in consumption order
                    oh_map = onehot_for(list(range(ci_base, ci_base + int(K_pass[ph]))))
                    sp = 0  # chunk cursor within pass stream
                    for w in range(W):
                        kf = int(k_fix[ph, w])
                        if kf == 0:
                            continue
                        lo = w * 128
                        cw = min(128, NC - lo)
                        aggs = [ps_agg.tile([128, 128], FP32, tag="agg", name=f"agg{m}")
                                for m in range(2)]
                        kf_eff = 1 if "aggmm" in ablate else kf
                        for j in range(kf_eff):
                            ci = sp + j
                            g = gtiles[ci // GI_CHUNKS]
                            if "NOCONS" in ablate:
                                gsl = gconst[:, :]
                            else:
                                gsl = g[:, ci % GI_CHUNKS, :]
                            oh, col0 = oh_map[ci_base + ci]
                            ohsl = oh[:, col0:col0 + 128]
                            for m in range(2):
                                mm = 0 if "G256B" in ablate else m
                                nc.tensor.matmul(
                                    aggs[m][:, :], gsl[:, 128 * mm:128 * (mm + 1)],
                                    ohsl, start=(j == 0), stop=(j == kf_eff - 1))
                        for m in range(2):
                            if ph == 0:
                                nc.vector.tensor_tensor(
                                    z_pre[m][:, lo:lo + cw], aggs[m][:, :cw],
                                    h_own[m][:, lo:lo + cw], op=ALU.add)
                            else:
                                nc.vector.tensor_tensor(
                                    z_pre[m][:, lo:lo + cw], aggs[m][:, :cw],
                                    z_pre[m][:, lo:lo + cw], op=ALU.add)
                        sp += kf
                    # windows with kf == 0 in pass A still need the h_own copy
                    if ph == 0:
                        for w in range(W):
                            if int(k_fix[0, w]) == 0:
                                lo = w * 128
                                cw = min(128, NC - lo)
                                for m in range(2):
                                    nc.vector.tensor_copy(z_pre[m][:, lo:lo + cw],
                                                          h_own[m][:, lo:lo + cw])

                # ---- MLP ----
                z2 = z_pre
                for jj in range(0 if "mlp" in ablate else MJ):
                    lo = jj * 512
                    nw = min(512, NC - lo)
                    z1t = [tpool.tile([128, 512], BF16, tag=f"z1_{m}", name=f"z1t{m}") for m in range(2)]
                    for m in range(2):
                        ps = ps_mlp.tile([128, 512], FP32, tag="mlp")
                        for k in range(2):
                            col = ((l * 2 + k) * 2 + m) * 128
                            nc.tensor.matmul(ps[:, :nw], w1_sb[:, col:col + 128],
                                             z_pre[k][:, lo:lo + nw],
                                             start=(k == 0), stop=(k == 1))
                        nc.scalar.activation(z1t[m][:, :nw], ps[:, :nw], AF.Relu,
                                             bias=b1_sb[:, 2 * l + m:2 * l + m + 1])
                    for m in range(2):
                        ps = ps_mlp.tile([128, 512], FP32, tag="mlp")
                        for k in range(2):
                            col = ((l * 2 + k) * 2 + m) * 128
                            nc.tensor.matmul(ps[:, :nw], w2_sb[:, col:col + 128],
                                             z1t[k][:, :nw],
                                             start=(k == 0), stop=(k == 1))
                        nc.scalar.activation(z2[m][:, lo:lo + nw], ps[:, :nw], AF.Identity,
                                             bias=b2_sb[:, 2 * l + m:2 * l + m + 1])

                # ---- BN stats + AllReduce ----
                nbch = _cdiv(NC, 512)
                stt = spool.tile([128, 4], FP32, tag="stt")
                bnacc = tpool.tile([128, nbch, 6], FP32, tag="bnacc")
                for m in range(2):
                    for jj in range(nbch):
                        lo = jj * 512
                        nw = min(512, NC - lo)
                        nc.vector.bn_stats(bnacc[:, jj, :], z2[m][:, lo:lo + nw])
                    ag = spool.tile([128, 2], FP32, tag="bnag")
                    nc.vector.bn_aggr(ag[:], bnacc[:])
                    # (mean, var) -> (mean, E[z^2])
                    sq = spool.tile([128, 1], FP32, tag="bnsq")
                    nc.vector.tensor_tensor(sq[:], ag[:, 0:1], ag[:, 0:1], op=ALU.mult)
                    nc.vector.tensor_copy(stt[:, 2 * m:2 * m + 1], ag[:, 0:1])
                    nc.vector.tensor_tensor(stt[:, 2 * m + 1:2 * m + 2], ag[:, 1:2],
                                            sq[:], op=ALU.add)
                nc.sync.dma_start(arbn_in[:], stt[:])
                if not skip_coll:
                    nc.gpsimd.collective_compute("AllReduce", ALU.add, replica_groups=RG,
                                                 ins=[arbn_in.opt()], outs=[arbn_out.opt()])
                stg = spool.tile([128, 4], FP32, tag="stg")
                nc.sync.dma_start(stg[:], arbn_out[:])
                # a = gamma * rsqrt(var + eps);  cb = beta - a * mean
                scl = spool.tile([128, 2], FP32, tag="scl")
                cbt = spool.tile([128, 2], FP32, tag="cbt")
                for m in range(2):
                    mean = spool.tile([128, 1], FP32, tag="bmean")
                    e2 = spool.tile([128, 1], FP32, tag="be2")
                    nc.vector.tensor_scalar(mean[:], stg[:, 2 * m:2 * m + 1], 0.125,
                                            None, op0=ALU.mult)
                    nc.vector.tensor_scalar(e2[:], stg[:, 2 * m + 1:2 * m + 2], 0.125,
                                            None, op0=ALU.mult)
                    var = spool.tile([128, 1], FP32, tag="bvar")
                    nc.vector.tensor_tensor(var[:], mean[:], mean[:], op=ALU.mult)
                    nc.vector.tensor_tensor(var[:], e2[:], var[:], op=ALU.subtract)
                    nc.vector.tensor_scalar(var[:], var[:], float(BN_EPS), None,
                                            op0=ALU.add)
                    std = spool.tile([128, 1], FP32, tag="bstd")
                    nc.scalar.activation(std[:], var[:], AF.Sqrt)
                    rstd = spool.tile([128, 1], FP32, tag="brstd")
                    nc.vector.reciprocal(rstd[:], std[:])
                    nc.vector.tensor_tensor(scl[:, m:m + 1], rstd[:],
                                            gam_sb[:, 2 * l + m:2 * l + m + 1], op=ALU.mult)
                    tmp = spool.tile([128, 1], FP32, tag="btmp")
                    nc.vector.tensor_tensor(tmp[:], scl[:, m:m + 1], mean[:], op=ALU.mult)
                    nc.vector.tensor_tensor(cbt[:, m:m + 1],
                                            bet_sb[:, 2 * l + m:2 * l + m + 1],
                                            tmp[:], op=ALU.subtract)

                # ---- normalize + relu -> h_next ----
                h_next = [hpool.tile([128, NC], BF16, tag=f"hown{m}", name=f"hnext{m}") for m in range(2)]
                for m in range(2):
                    for jj in range(nbch):
                        lo = jj * 512
                        nw = min(512, NC - lo)
                        nc.scalar.activation(h_next[m][:, lo:lo + nw],
                                             z2[m][:, lo:lo + nw], AF.Relu,
                                             bias=cbt[:, m:m + 1], scale=scl[:, m:m + 1])

                # ---- transpose to node-major; pooling; slice write ----
                pooled_ps = ps_msc.tile([128, 256], FP32, tag="msc")
                for j in range(NCH):
                    lo = j * 128
                    cw = min(128, NC - lo)
                    hnm = tpool.tile([128, 256], BF16, tag="hnm")
                    for m in range(2):
                        tp = ps_msc.tile([128, 128], BF16, tag="msc")
                        nc.tensor.transpose(tp[:cw, :], h_next[m][:, lo:lo + cw],
                                            ident[:])
                        nc.vector.tensor_copy(hnm[:cw, 128 * m:128 * (m + 1)],
                                              tp[:cw, :])
                    # pooling one-hot + matmul
                    po = ohpool.tile([128, 128], BF16, tag="poh")
                    nc.vector.tensor_tensor(
                        po[:], iota_sb[:, :128],
                        batchloc_sb[:, j:j + 1].to_broadcast([128, 128]),
                        op=ALU.is_equal)
                    nc.tensor.matmul(pooled_ps[:], po[:cw, :], hnm[:cw, :],
                                     start=(j == 0), stop=(j == NCH - 1))
                    if l < L - 1:
                        nc.sync.dma_start(sliceM[lo:lo + cw, :], hnm[:cw, :])
                pooled_sb = tpool.tile([128, 256], BF16, tag="pooled")
                nc.vector.tensor_copy(pooled_sb[:], pooled_ps[:])
                for m in range(2):
                    pl_ps = ps_msc.tile([128, 512], FP32, tag="msc")
                    nc.tensor.matmul(pl_ps[:], pooled_sb[:, 128 * m:128 * (m + 1)],
                                     placem_sb[:], start=True, stop=True)
                    gp = tpool.tile([128, 512], FP32, tag="gp")
                    nc.vector.tensor_copy(gp[:], pl_ps[:])
                    nc.sync.dma_start(arp_in[(l * 2 + m) * 128:(l * 2 + m + 1) * 128, :],
                                      gp[:])
                h_own = h_next

            # ---------------- pooled AllReduce + head ---------------------------
            if not skip_coll:
                nc.gpsimd.collective_compute("AllReduce", ALU.add, replica_groups=RG,
                                             ins=[arp_in.opt()], outs=[arp_out.opt()])
            y1ps = ps_mlp.tile([128, 512], FP32, tag="mlp")
            gtiles = []
            for k in range(DL // 128):
                gk = tpool.tile([128, 512], FP32, tag="gark")
                nc.sync.dma_start(gk[:], arp_out[128 * k:128 * (k + 1), :])
                gtiles.append(gk)
            for k in range(DL // 128):
                nc.tensor.matmul(y1ps[:], wfc1_sb[:, 128 * k:128 * (k + 1)],
                                 gtiles[k][:], start=(k == 0), stop=(k == DL // 128 - 1))
            y1 = tpool.tile([128, 512], FP32, tag="y1")
            nc.scalar.activation(y1[:], y1ps[:], AF.Relu, bias=bfc1_sb[:])
            y2ps = ps_msc.tile([1, 512], FP32, tag="msc")
            nc.tensor.matmul(y2ps[:], wfc2_sb[:], y1[:], start=True, stop=True)
            osb = tpool.tile([1, 512], FP32, tag="osb")
            nc.scalar.activation(osb[:], y2ps[:], AF.Identity, bias=bfc2_sb[:])
            nc.sync.dma_start(out_d[:], osb[:])

        if reps and straight:
            for _ in range(reps):
                emit_body()
        elif reps:
            with tc.For_i(0, reps, 1):
                emit_body()
        else:
            emit_body()

    nc.compile()
    return nc


# ==================================================================== run
_CACHE = {}


def _get_runner(p):
    import jax
    from jax.sharding import Mesh, PartitionSpec
    from jax.experimental.shard_map import shard_map
    from concourse.bass2jax import _bass_exec_p, install_neuronx_cc_hook

    nc = build_program(p)
    install_neuronx_cc_hook()
    part_name = nc.partition_id_tensor.name if nc.partition_id_tensor else None
    in_names, out_names, out_avals, zero_outs = [], [], [], []
    for alloc in nc.m.functions[0].allocations:
        if not isinstance(alloc, mybir.MemoryLocationSet):
            continue
        name = alloc.memorylocations[0].name
        if alloc.kind == "ExternalInput":
            if name != part_name:
                in_names.append(name)
        elif alloc.kind == "ExternalOutput":
            out_names.append(name)
            shape = tuple(alloc.tensor_shape)
            dtype = mybir.dt.np(alloc.dtype)
            out_avals.append(jax.core.ShapedArray(shape, dtype))
            zero_outs.append(np.zeros(shape, dtype))
    n_params = len(in_names)
    all_in_names = list(in_names) + list(out_names)
    if part_name is not None:
        all_in_names.append(part_name)

    def _body(*args):
        from concourse.bass2jax import partition_id_tensor
        operands = list(args)
        if part_name is not None:
            operands.append(partition_id_tensor())
        outs = _bass_exec_p.bind(
            *operands, out_avals=tuple(out_avals), in_names=tuple(all_in_names),
            out_names=tuple(out_names), lowering_input_output_aliases=(),
            sim_require_finite=False, sim_require_nnan=False, nc=nc)
        return tuple(outs)

    devices = jax.devices()[:N_CORES]
    mesh = Mesh(np.asarray(devices), ("core",))
    specs = (PartitionSpec("core"),) * (n_params + len(out_names))
    fn = jax.jit(shard_map(_body, mesh=mesh, in_specs=specs,
                           out_specs=(PartitionSpec("core"),) * len(out_names),
                           check_rep=False), keep_unused=True)
    return nc, fn, in_names, out_names, out_avals, zero_outs, mesh


def _device_args(p):
    import jax
    from jax.sharding import NamedSharding, PartitionSpec
    nc, fn, in_names, out_names, out_avals, zero_outs, mesh = _CACHE["runner"]
    per_core_maps = []
    for c in range(N_CORES):
        m = dict(p.shared)
        m.update(p.per_core[c])
        per_core_maps.append(m)
    concat_in = [np.concatenate([np.asarray(per_core_maps[c][nm])[None]
                                 for c in range(N_CORES)], axis=0)
                 .reshape(-1, *np.asarray(per_core_maps[0][nm]).shape[1:])
                 for nm in in_names]
    concat_zero = [np.zeros((N_CORES * z.shape[0], *z.shape[1:]), z.dtype)
                   for z in zero_outs]
    sh = NamedSharding(mesh, PartitionSpec("core"))
    args = [jax.device_put(a, sh) for a in concat_in + concat_zero]
    for a in args:
        a.block_until_ready()
    return args


def run_on_device(p):
    import jax
    sig = (p.N, p.E, p.G, p.Ktot, tuple(map(int, p.K_pass)),
           tuple(map(int, p.k_fix.ravel())))
    if _CACHE.get("sig") != sig:
        _CACHE.clear()
        _CACHE["sig"] = sig
    if "runner" not in _CACHE:
        _CACHE["runner"] = _get_runner(p)
    if "args" not in _CACHE:
        _CACHE["args"] = _device_args(p)
    nc, fn, in_names, out_names, out_avals, zero_outs, mesh = _CACHE["runner"]
    outs = fn(*_CACHE["args"])
    for o in outs:
        o.block_until_ready()
    res = np.asarray(outs[out_names.index("out")])
    res = res.reshape(N_CORES, 1, 512)[0, 0]     # core 0
    return res


def kernel(**inputs):
    p = preprocess(**inputs)
    _CACHE.pop("args", None)       # force fresh input upload for new data
    out = run_on_device(p)
    return out[:p.G].astype(np.float32).reshape(p.G, 1)

